# revision 5
# baseline (speedup 1.0000x reference)
"""Trainium2 Bass kernel for EvolutionGeneratorLognormal.

Computes logsamples = cumsum_dates(einsum('nij,njs->nis', cov, z) - var/2)
for cov [252,8,8], var [252,8], z [252,8,65536] -> out [252,8,65536] f32.

Strategy (per core, sims sharded 8 ways -> 8192 sims/core):
  - Dates split into 15 groups of 16 dates + a final group of 12 dates
    (252 = 15*16 + 12).  Within a group the (date, factor) pairs occupy
    the SBUF partitions, with dates REVERSED so the group's last date
    sits at partitions 0:8.
  - One block-lower-triangular matmul per (group, sim-chunk) computes
    the within-group einsum AND within-group date-cumsum at once.
  - A second K=8 matmul accumulates the running carry (previous group's
    last-date rows of the OUTPUT tile) broadcast to all date blocks,
    into the same PSUM bank.
  - The -0.5*cumsum(var) term is folded into the PSUM->SBUF copy as a
    per-partition tensor_scalar subtract.  The output tile's first 8
    partitions double as the next group's carry matmul operand -- no
    separate carry tile/op.
  - z / cov / output travel as fp16 (halves HBM traffic vs f32; PSUM
    accumulation stays f32; measured end-to-end rel err ~1.8e-3).
    Host casts the fp16 output back to f32.
  - Sim chunks of 512 (one fp32 PSUM bank); 16 chunks cover 8192 sims.
"""

import sys

sys.path.insert(0, "/opt/trn_rl_repo")

import numpy as np

import concourse.bacc as bacc
import concourse.mybir as mybir
import concourse.tile as tile
from concourse.bass_utils import run_bass_kernel_spmd

N_DATES = 252
M = 8
N_SIMS = 65536
N_CORES = 8
SC = N_SIMS // N_CORES          # sims per core
G = 16                          # date groups
DG = 16                         # dates per full group
DG_LAST = N_DATES - (G - 1) * DG        # 12 dates in the last group
P = 128                         # partitions = DG * M
P_LAST = DG_LAST * M            # 96 partitions in the last group
CH = 512                        # sim chunk (one fp32 PSUM bank)
NCH = SC // CH

F32 = mybir.dt.float32
F32R = mybir.dt.float32r
F16 = mybir.dt.float16

_CACHED = {}


def _grp_rows(g):
    return P_LAST if g == G - 1 else P


def _build_nc(reps=1):
    nc = bacc.Bacc(trn_type="TRN2", debug=False, num_devices=N_CORES)
    z_d = nc.dram_tensor("z", (G * P, SC), F16, kind="ExternalInput")
    lt_d = nc.dram_tensor("lt", (G, P, P), F16, kind="ExternalInput")
    vrel_d = nc.dram_tensor("vrel", (P, G), F32, kind="ExternalInput")
    id8_d = nc.dram_tensor("id8", (M, P), F16, kind="ExternalInput")
    out_d = nc.dram_tensor("out", (G * P, SC), F16, kind="ExternalOutput")

    with tile.TileContext(nc) as tc:
        with (
            tc.tile_pool(name="const", bufs=1) as constp,
            tc.tile_pool(name="zp", bufs=3) as zp,
            tc.tile_pool(name="op", bufs=2) as op,
            tc.tile_pool(name="ps", bufs=8, space="PSUM") as psp,
        ):
            lt_t = constp.tile([P, G, P], F16)
            nc.sync.dma_start(lt_t[:], lt_d.ap().rearrange("g p m -> p g m"))
            vrel_t = constp.tile([P, G], F32)
            nc.sync.dma_start(vrel_t[:], vrel_d.ap())
            id8_t = constp.tile([M, P], F16)
            nc.sync.dma_start(id8_t[:], id8_d.ap())

            for _rep in range(reps):
                prev_ot = None
                for g in range(G):
                    R = _grp_rows(g)
                    zt = zp.tile([P, SC], F16)
                    nc.sync.dma_start(
                        zt[0:R, :], z_d.ap()[g * P:g * P + R, :]
                    )
                    ot = op.tile([P, SC], F16)
                    for k in range(NCH):
                        ps = psp.tile([P, CH], F32)
                        ck = slice(k * CH, (k + 1) * CH)
                        nc.tensor.matmul(
                            ps[0:R, :],
                            lt_t[0:R, g, 0:R],
                            zt[0:R, ck],
                            start=True,
                            stop=(prev_ot is None),
                        )
                        if prev_ot is not None:
                            nc.tensor.matmul(
                                ps[0:R, :],
                                id8_t[:, 0:R],
                                prev_ot[0:M, ck],
                                start=False,
                                stop=True,
                            )
                        nc.vector.tensor_scalar_sub(
                            ot[0:R, ck], ps[0:R, :], vrel_t[0:R, g:g + 1]
                        )
                    # stores on the ACT HWDGE ring so they don't queue behind
                    # the next group's z load on the SP ring
                    nc.scalar.dma_start(
                        out_d.ap()[g * P:g * P + R, :], ot[0:R, :]
                    )
                    prev_ot = ot

    nc.compile()
    return nc


def _host_prep(cov, var, z):
    """Build per-core kernel inputs in the (group, reversed-date) layout.

    Group g < 15 holds dates [16g, 16g+16), group 15 holds dates
    [240, 252).  Within a group, partition row r*8+i (r reversed) maps
    to date base + (dg-1-r), factor i, where dg is the group's size.
    """
    # Block-lower-triangular cumsum matrices, one per group, in the
    # reversed-date basis: lt[g, rp*8+j, r*8+i] = cov[base+dp, i, j] for
    # dp <= dc  (rp = dg-1-dp >= r = dg-1-dc).
    lt = np.zeros((G, P, P), np.float16)
    for g in range(G):
        dg = DG_LAST if g == G - 1 else DG
        covg = cov[g * DG:g * DG + dg]             # [dg, M, M], index dp
        for dp in range(dg):
            rp = dg - 1 - dp
            blk = covg[dp].T                       # [j, i]
            for dc in range(dp, dg):
                r = dg - 1 - dc
                lt[g, rp * M:(rp + 1) * M, r * M:(r + 1) * M] = blk

    # vrel[g, r*8+i] = 0.5 * (cumvar[base + (dg-1-r), i] - cumvar[base-1, i])
    cumvar = 0.5 * np.cumsum(var, axis=0)          # [N_DATES, M]
    vrel = np.zeros((G, DG, M), np.float32)
    for g in range(G):
        dg = DG_LAST if g == G - 1 else DG
        base = cumvar[g * DG - 1] if g > 0 else np.zeros(M, np.float32)
        for r in range(dg):
            d = dg - 1 - r
            vrel[g, r] = cumvar[g * DG + d] - base
    vrel_pm = np.ascontiguousarray(vrel.reshape(G, P).T)   # [P, G]

    # id8[j, r*8+i] = (i == j): broadcast carry rows to all date blocks
    id8 = np.zeros((M, P), np.float16)
    for j in range(M):
        id8[j, j::M] = 1.0

    # z in kernel layout: [G, dg (reversed), M, sims]; rows beyond the
    # last group's 96 are never touched by the kernel.
    zq = z.reshape(N_DATES, M, N_SIMS)
    in_maps = []
    for c in range(N_CORES):
        zc = np.empty((G * P, SC), np.float16)
        for g in range(G):
            dg = DG_LAST if g == G - 1 else DG
            blk = zq[g * DG:g * DG + dg, :, c * SC:(c + 1) * SC][::-1]
            zc[g * P:g * P + dg * M] = blk.reshape(dg * M, SC)
        in_maps.append({"z": zc, "lt": lt, "vrel": vrel_pm, "id8": id8})
    return in_maps


def _host_gather(results):
    fin = np.empty((N_DATES, M, N_SIMS), np.float32)
    for c in range(N_CORES):
        oc = results[c]["out"]
        cs = slice(c * SC, (c + 1) * SC)
        for g in range(G):
            dg = DG_LAST if g == G - 1 else DG
            blk = oc[g * P:g * P + dg * M].reshape(dg, M, SC)
            fin[g * DG:g * DG + dg, :, cs] = blk[::-1]
    return fin


def kernel(cov, var, z, _reps=1):
    cov = np.asarray(cov, dtype=np.float32)
    var = np.asarray(var, dtype=np.float32)
    z = np.asarray(z, dtype=np.float32)
    if _reps not in _CACHED:
        _CACHED[_reps] = _build_nc(_reps)
    nc = _CACHED[_reps]
    in_maps = _host_prep(cov, var, z)
    res = run_bass_kernel_spmd(nc, in_maps, core_ids=list(range(N_CORES)))
    return _host_gather(res.results)


# revision 12
# speedup vs baseline: 1.3512x; 1.3512x over previous
"""Trainium2 Bass kernel for EvolutionGeneratorLognormal.

Computes logsamples = cumsum_dates(einsum('nij,njs->nis', cov, z) - var/2)
for cov [252,8,8], var [252,8], z [252,8,65536] -> out [252,8,65536] f32.

Strategy (per core, sims sharded 8 ways -> 8192 sims/core):
  - Dates split into 15 groups of 16 dates + a final group of 12 dates
    (252 = 15*16 + 12).  Within a group the (date, factor) pairs occupy
    the SBUF partitions, with dates REVERSED so the group's last date
    sits at partitions 0:8.
  - One block-lower-triangular matmul per (group, sim-chunk) computes
    the within-group einsum AND within-group date-cumsum at once.
  - A second K=8 matmul accumulates the running carry (previous group's
    last-date rows of the OUTPUT tile) broadcast to all date blocks,
    into the same PSUM bank.
  - The -0.5*cumsum(var) term is folded into the PSUM->SBUF copy as a
    per-partition tensor_scalar subtract.  The output tile's first 8
    partitions double as the next group's carry matmul operand -- no
    separate carry tile/op.
  - z / cov / output travel as fp16 (halves HBM traffic vs f32; PSUM
    accumulation stays f32; measured end-to-end rel err ~1.8e-3).
    Host casts the fp16 output back to f32.
  - Sim chunks of 512 (one fp32 PSUM bank); 16 chunks cover 8192 sims.
"""

import sys

sys.path.insert(0, "/opt/trn_rl_repo")

import numpy as np

import concourse.bacc as bacc
import concourse.mybir as mybir
import concourse.tile as tile
from concourse.bass_utils import run_bass_kernel_spmd

N_DATES = 252
M = 8
N_SIMS = 65536
N_CORES = 8
SC = N_SIMS // N_CORES          # sims per core
G = 16                          # date groups
DG = 16                         # dates per full group
DG_LAST = N_DATES - (G - 1) * DG        # 12 dates in the last group
P = 128                         # partitions = DG * M
P_LAST = DG_LAST * M            # 96 partitions in the last group
CH = 512                        # sim chunk (one fp32 PSUM bank)
NCH = SC // CH

F32 = mybir.dt.float32
F32R = mybir.dt.float32r
F16 = mybir.dt.float16

_CACHED = {}


def _grp_rows(g):
    return P_LAST if g == G - 1 else P


def _build_nc(reps=1):
    nc = bacc.Bacc(trn_type="TRN2", debug=False, num_devices=N_CORES)
    z_d = nc.dram_tensor("z", (G * P, SC), F16, kind="ExternalInput")
    lt_d = nc.dram_tensor("lt", (G, P, P), F16, kind="ExternalInput")
    nvrel_d = nc.dram_tensor("nvrel", (P, G), F32, kind="ExternalInput")
    id8_d = nc.dram_tensor("id8", (M, P), F16, kind="ExternalInput")
    out_d = nc.dram_tensor("out", (G * P, SC), F16, kind="ExternalOutput")

    with tile.TileContext(nc) as tc:
        with (
            tc.tile_pool(name="const", bufs=1) as constp,
            tc.tile_pool(name="zp", bufs=3) as zp,
            tc.tile_pool(name="op", bufs=2) as op,
            tc.tile_pool(name="ps", bufs=8, space="PSUM") as psp,
        ):
            lt_t = constp.tile([P, G, P], F16)
            nc.sync.dma_start(lt_t[:], lt_d.ap().rearrange("g p m -> p g m"))
            nvrel_t = constp.tile([P, G], F32)
            nc.sync.dma_start(nvrel_t[:], nvrel_d.ap())
            id8_t = constp.tile([M, P], F16)
            nc.sync.dma_start(id8_t[:], id8_d.ap())

            for _rep in range(reps):
                prev_ot = None
                for g in range(G):
                    R = _grp_rows(g)
                    zt = zp.tile([P, SC], F16)
                    nc.sync.dma_start(
                        zt[0:R, :], z_d.ap()[g * P:g * P + R, :]
                    )
                    ot = op.tile([P, SC], F16)
                    for k in range(NCH):
                        ps = psp.tile([P, CH], F32)
                        ck = slice(k * CH, (k + 1) * CH)
                        nc.tensor.matmul(
                            ps[0:R, :],
                            lt_t[0:R, g, 0:R],
                            zt[0:R, ck],
                            start=True,
                            stop=(prev_ot is None),
                        )
                        if prev_ot is not None:
                            nc.tensor.matmul(
                                ps[0:R, :],
                                id8_t[:, 0:R],
                                prev_ot[0:M, ck],
                                start=False,
                                stop=True,
                            )
                        # PSUM->SBUF drain (+ -0.5*cumvar bias) split across
                        # DVE and ACT so neither engine bottlenecks
                        # (GPSIMD cannot read PSUM on TRN2)
                        bias = nvrel_t[0:R, g:g + 1]
                        if k < 10:
                            nc.vector.tensor_scalar_add(
                                ot[0:R, ck], ps[0:R, :], bias
                            )
                        else:
                            nc.scalar.activation(
                                ot[0:R, ck], ps[0:R, :],
                                mybir.ActivationFunctionType.Identity,
                                bias=bias,
                            )
                    # stores on the ACT HWDGE ring so they don't queue behind
                    # the next group's z load on the SP ring
                    nc.scalar.dma_start(
                        out_d.ap()[g * P:g * P + R, :], ot[0:R, :]
                    )
                    prev_ot = ot

    nc.compile()
    return nc


def _host_prep(cov, var, z):
    """Build per-core kernel inputs in the (group, reversed-date) layout.

    Group g < 15 holds dates [16g, 16g+16), group 15 holds dates
    [240, 252).  Within a group, partition row r*8+i (r reversed) maps
    to date base + (dg-1-r), factor i, where dg is the group's size.
    """
    # Block-lower-triangular cumsum matrices, one per group, in the
    # reversed-date basis: lt[g, rp*8+j, r*8+i] = cov[base+dp, i, j] for
    # dp <= dc  (rp = dg-1-dp >= r = dg-1-dc).
    lt = np.zeros((G, P, P), np.float16)
    for g in range(G):
        dg = DG_LAST if g == G - 1 else DG
        covg = cov[g * DG:g * DG + dg]             # [dg, M, M], index dp
        for dp in range(dg):
            rp = dg - 1 - dp
            blk = covg[dp].T                       # [j, i]
            for dc in range(dp, dg):
                r = dg - 1 - dc
                lt[g, rp * M:(rp + 1) * M, r * M:(r + 1) * M] = blk

    # vrel[g, r*8+i] = 0.5 * (cumvar[base + (dg-1-r), i] - cumvar[base-1, i])
    cumvar = 0.5 * np.cumsum(var, axis=0)          # [N_DATES, M]
    vrel = np.zeros((G, DG, M), np.float32)
    for g in range(G):
        dg = DG_LAST if g == G - 1 else DG
        base = cumvar[g * DG - 1] if g > 0 else np.zeros(M, np.float32)
        for r in range(dg):
            d = dg - 1 - r
            vrel[g, r] = cumvar[g * DG + d] - base
    nvrel_pm = np.ascontiguousarray(-vrel.reshape(G, P).T)   # [P, G]

    # id8[j, r*8+i] = (i == j): broadcast carry rows to all date blocks
    id8 = np.zeros((M, P), np.float16)
    for j in range(M):
        id8[j, j::M] = 1.0

    # z in kernel layout: [G, dg (reversed), M, sims]; rows beyond the
    # last group's 96 are never touched by the kernel.
    zq = z.reshape(N_DATES, M, N_SIMS)
    in_maps = []
    for c in range(N_CORES):
        zc = np.empty((G * P, SC), np.float16)
        for g in range(G):
            dg = DG_LAST if g == G - 1 else DG
            blk = zq[g * DG:g * DG + dg, :, c * SC:(c + 1) * SC][::-1]
            zc[g * P:g * P + dg * M] = blk.reshape(dg * M, SC)
        in_maps.append({"z": zc, "lt": lt, "nvrel": nvrel_pm, "id8": id8})
    return in_maps


def _host_gather(results):
    fin = np.empty((N_DATES, M, N_SIMS), np.float32)
    for c in range(N_CORES):
        oc = results[c]["out"]
        cs = slice(c * SC, (c + 1) * SC)
        for g in range(G):
            dg = DG_LAST if g == G - 1 else DG
            blk = oc[g * P:g * P + dg * M].reshape(dg, M, SC)
            fin[g * DG:g * DG + dg, :, cs] = blk[::-1]
    return fin


def kernel(cov, var, z, _reps=1):
    cov = np.asarray(cov, dtype=np.float32)
    var = np.asarray(var, dtype=np.float32)
    z = np.asarray(z, dtype=np.float32)
    if _reps not in _CACHED:
        _CACHED[_reps] = _build_nc(_reps)
    nc = _CACHED[_reps]
    in_maps = _host_prep(cov, var, z)
    res = run_bass_kernel_spmd(nc, in_maps, core_ids=list(range(N_CORES)))
    return _host_gather(res.results)
